# revision 1
# baseline (speedup 1.0000x reference)
"""Trainium2 Bass kernel for LISTA (nn_LISTA_37976100831401).

Data-parallel sharding: batch 16384 -> 8 NeuronCores x 2048 rows.
We / S / theta are replicated on every core; there is no cross-device
communication in the forward pass.

Per-core algorithm:
  B  = X @ We.T                 (2048, 1024)
  Z0 = soft(B);  Z_{t+1} = soft(B + Z_t @ S.T), t = 0..15
  soft(x) = relu(x - theta) - relu(-x - theta)

Host-side marshaling ships X.T / We.T / S.T per core, so every matmul
operand DMA-loads directly in its on-chip layout (contraction dim on
partitions) -- zero PE transposes.  Z.T keeps the feature dim m=1024 on SBUF
partitions (8 tiles of 128) and batch on the free dim; then
C.T = S @ Z.T + B.T accumulates in PSUM via
  psum[j,b] += ST[k][:, j128].T @ ZT[k][:, b]        (lhsT = S.T tile)
and the matmul OUTPUT layout [j, b] equals the INPUT layout [k, b] the next
step needs.  All matmuls run in fp32r (1 PE cycle/row for moving dim >= 256,
vs 4 cycles/row for plain fp32).

The final step is computed in the flipped orientation [b, j] (stationary =
Z.T columns, moving = S.T) with the X@We.T term accumulated in the same PSUM
group, so the result lands batch-major and DMAs out contiguously.
"""

import time
from contextlib import ExitStack

import numpy as np

import concourse.bacc as bacc
import concourse.mybir as mybir
import concourse.tile as tile
from concourse import bass_utils
from concourse.masks import make_identity

FP32 = mybir.dt.float32
FP32R = mybir.dt.float32r
AL = mybir.AluOpType
AF = mybir.ActivationFunctionType

N_CORES = 8
B_TOTAL, N_IN, M = 16384, 256, 1024
B_CORE = B_TOTAL // N_CORES  # 2048
T_STEPS = 16                 # scan length in the reference
CHUNK = 512                  # batch columns per j-sweep (= PSUM bank / max N)
N_CHUNKS = B_CORE // CHUNK   # 4
KT = M // 128                # 8 feature tiles of 128
NT = N_IN // 128             # 2 input-feature tiles
JHN = M // CHUNK             # 2 output-column halves in the final step


def _emit(ctx: ExitStack, tc: tile.TileContext, XT, WeT, ST, theta, Z):
    nc = tc.nc

    const_pool = ctx.enter_context(tc.tile_pool(name="const", bufs=1))
    psum_pool = ctx.enter_context(tc.tile_pool(name="psum", bufs=1, space="PSUM"))
    xt_pool = ctx.enter_context(tc.tile_pool(name="xt", bufs=1))
    bt_pool = ctx.enter_context(tc.tile_pool(name="bt", bufs=1))
    zt_pool = ctx.enter_context(tc.tile_pool(name="zt", bufs=1))
    tmp_pool = ctx.enter_context(tc.tile_pool(name="tmp", bufs=1))
    out_pool = ctx.enter_context(tc.tile_pool(name="zout", bufs=1))

    # ---- constants -------------------------------------------------------
    ident = const_pool.tile([128, 128], FP32, name="ident")
    make_identity(nc, ident[:])
    th = const_pool.tile([128, KT], FP32, name="th")
    nth = const_pool.tile([128, KT], FP32, name="nth")
    wet = [const_pool.tile([128, M], FP32R, name=f"wet{nt}") for nt in range(NT)]
    st = [const_pool.tile([128, M], FP32R, name=f"st{kt}") for kt in range(KT)]
    th_row = const_pool.tile([1, M], FP32, name="th_row")
    thbc = const_pool.tile([128, M], FP32, name="thbc")
    nthbc = const_pool.tile([128, M], FP32, name="nthbc")

    def emit_th():
        # Only the contiguous theta row comes from DRAM (4KB, 1 descriptor),
        # first on the otherwise-idle scalar ring so the theta-broadcast
        # matmuls become the PE's earliest possible work.  A strided theta
        # DMA would emit 1024 4-byte descriptors and block a ring for ~10us;
        # th is derived on-chip instead (emit_th_derive).
        nc.scalar.dma_start(th_row[:], theta.rearrange("(o m) -> o m", o=1))

    def emit_th_derive():
        # th[p, jt] = theta[jt*128 + p]: transpose a [128,128] slab of thbc
        # (constant along partitions) -- every column of the result is the
        # wanted theta block; copy out column 0.
        for jt in range(KT):
            pt = psum_pool.tile([128, 128], FP32, name="ptth", tag="tp", bufs=2)
            nc.tensor.transpose(
                pt[:], thbc[:, jt * 128 : (jt + 1) * 128], ident[:]
            )
            nc.vector.tensor_copy(th[:, jt : jt + 1], pt[:, 0:1])
        nc.vector.tensor_scalar_mul(nth[:], th[:], -1.0)

    def emit_we():
        for nt in range(NT):
            nc.sync.dma_start(wet[nt][:], WeT[nt * 128 : (nt + 1) * 128, :])

    def emit_st():
        # S.T row-blocks straight into the stationary tiles, split across
        # both HWDGE queues.
        for kt in range(KT):
            dma_eng = nc.scalar if kt % 2 == 0 else nc.sync
            dma_eng.dma_start(st[kt][:], ST[kt * 128 : (kt + 1) * 128, :])

    def emit_thbc():
        # theta broadcast across partitions (for the flipped final step):
        # thbc[p, j] = theta[j], built with a K=1 ones-matmul.
        ones_col = const_pool.tile([1, 128], FP32, name="ones_col")
        nc.gpsimd.memset(ones_col[:], 1.0)
        for jh in range(JHN):
            pbc = psum_pool.tile([128, CHUNK], FP32, name="pbc", tag="tp", bufs=2)
            nc.tensor.matmul(
                pbc[:], ones_col[:], th_row[:, jh * CHUNK : (jh + 1) * CHUNK],
                start=True, stop=True,
            )
            nc.vector.tensor_copy(thbc[:, jh * CHUNK : (jh + 1) * CHUNK], pbc[:])
        nc.vector.tensor_scalar_mul(nthbc[:], thbc[:], -1.0)

    # ---- per-chunk state -------------------------------------------------
    xts = {}  # chunk -> [NT] tiles [128, CHUNK]   (X.T slab)
    bts = {}  # chunk -> [KT] tiles [128, CHUNK]   (B.T slab)
    zts = {}  # chunk -> [KT] tiles [128, CHUNK]   (current Z.T)

    def x_phase(c, dma_eng=None):
        dma_eng = dma_eng or nc.sync
        xts[c] = [
            xt_pool.tile([128, CHUNK], FP32R, name=f"xt{nt}", tag=f"xt{nt}", bufs=4)
            for nt in range(NT)
        ]
        for nt in range(NT):
            dma_eng.dma_start(
                xts[c][nt][:],
                XT[nt * 128 : (nt + 1) * 128, c * CHUNK : (c + 1) * CHUNK],
            )

    def b_tile(c, jt, head_split=False):
        # One tile of: B.T = We @ X.T ; Z0 = soft(B).  The threshold ops read
        # PSUM directly; the B copy runs on ACT in parallel.
        ps = psum_pool.tile([128, CHUNK], FP32, name="psb", tag="mm", bufs=6)
        for nt in range(NT):
            nc.tensor.matmul(
                ps[:],
                wet[nt][:, jt * 128 : (jt + 1) * 128],
                xts[c][nt][:],
                start=(nt == 0),
                stop=(nt == NT - 1),
            )
        btile = bt_pool.tile(
            [128, CHUNK], FP32, name="btile", tag=f"bt{jt}", bufs=2
        )
        z0 = zt_pool.tile([128, CHUNK], FP32R, name="z0", tag=f"zt{jt}", bufs=3)
        if head_split and jt % 2 == 1:
            # Head-critical chunk: odd tiles take the DVE dual-op path so the
            # ACT and DVE chains for Z0 run in parallel.
            # soft(B) = max(B-th, 0) + min(B+th, 0)
            af = tmp_pool.tile([128, CHUNK], FP32, name="afb", tag="af", bufs=3)
            nc.vector.tensor_scalar(
                af[:], ps[:], th[:, jt : jt + 1], 0.0, op0=AL.subtract, op1=AL.max
            )
            df = tmp_pool.tile([128, CHUNK], FP32, name="dfb", tag="df", bufs=3)
            nc.vector.tensor_scalar(
                df[:], ps[:], th[:, jt : jt + 1], 0.0, op0=AL.add, op1=AL.min
            )
            nc.scalar.copy(btile[:], ps[:])
            nc.vector.tensor_add(z0[:], af[:], df[:])
        else:
            af = tmp_pool.tile([128, CHUNK], FP32, name="afb", tag="af", bufs=3)
            nc.scalar.activation(
                af[:], ps[:], AF.Relu, bias=nth[:, jt : jt + 1], scale=1.0
            )
            df = tmp_pool.tile([128, CHUNK], FP32, name="dfb", tag="df", bufs=3)
            nc.scalar.activation(
                df[:], ps[:], AF.Relu, bias=nth[:, jt : jt + 1], scale=-1.0
            )
            nc.scalar.copy(btile[:], ps[:])
            nc.vector.tensor_sub(z0[:], af[:], df[:])
        bts[c].append(btile)
        zts[c].append(z0)

    def b_phase(c, head_split=False):
        bts[c] = []
        zts[c] = []
        for jt in range(KT):
            b_tile(c, jt, head_split=head_split)

    def step(c):
        # Z <- soft(B + Z @ S.T), in the [j, b] orientation.
        zcur = zts[c]
        znew = []
        for jt in range(KT):
            ps = psum_pool.tile([128, CHUNK], FP32, name="pss", tag="mm", bufs=6)
            for kt in range(KT):
                nc.tensor.matmul(
                    ps[:],
                    st[kt][:, jt * 128 : (jt + 1) * 128],
                    zcur[kt][:],
                    start=(kt == 0),
                    stop=(kt == KT - 1),
                )
            ct = tmp_pool.tile([128, CHUNK], FP32, name="ct", tag="ct", bufs=3)
            nc.vector.tensor_add(ct[:], ps[:], bts[c][jt][:])
            af = tmp_pool.tile([128, CHUNK], FP32, name="afs", tag="af", bufs=3)
            nc.scalar.activation(
                af[:], ct[:], AF.Relu, bias=nth[:, jt : jt + 1], scale=1.0
            )
            df = tmp_pool.tile([128, CHUNK], FP32, name="dfs", tag="df", bufs=3)
            nc.scalar.activation(
                df[:], ct[:], AF.Relu, bias=nth[:, jt : jt + 1], scale=-1.0
            )
            zn = zt_pool.tile([128, CHUNK], FP32R, name="zn", tag=f"zt{jt}", bufs=3)
            nc.vector.tensor_sub(zn[:], af[:], df[:])
            znew.append(zn)
        zts[c] = znew

    def final_tile(c, idx):
        # Last step in flipped orientation: out[b, j], so the store DMA is
        # contiguous along DRAM rows.  C = Z@S.T + X@We.T accumulated in PSUM.
        row0 = c * CHUNK
        bt, jh = divmod(idx, JHN)
        zcur = zts[c]
        jsl = slice(jh * CHUNK, (jh + 1) * CHUNK)
        ps = psum_pool.tile([128, CHUNK], FP32, name="psf", tag="mm", bufs=6)
        for kt in range(KT):
            nc.tensor.matmul(
                ps[:],
                zcur[kt][:, bt * 128 : (bt + 1) * 128],
                st[kt][:, jsl],
                start=(kt == 0),
                stop=False,
            )
        for nt in range(NT):
            nc.tensor.matmul(
                ps[:],
                xts[c][nt][:, bt * 128 : (bt + 1) * 128],
                wet[nt][:, jsl],
                start=False,
                stop=(nt == NT - 1),
            )
        t1 = tmp_pool.tile([128, CHUNK], FP32, name="t1f", tag="ct", bufs=3)
        nc.vector.tensor_max(t1[:], ps[:], nthbc[:, jsl])
        t2 = tmp_pool.tile([128, CHUNK], FP32, name="t2f", tag="tcl", bufs=2)
        nc.vector.tensor_tensor(t2[:], t1[:], thbc[:, jsl], op=AL.min)
        zo = out_pool.tile([128, CHUNK], FP32, name="zo", tag="zo", bufs=3)
        nc.vector.tensor_sub(zo[:], ps[:], t2[:])
        nc.sync.dma_start(Z[row0 + bt * 128 : row0 + (bt + 1) * 128, jsl], zo[:])

    # ---- emission schedule: all operands DMA straight into place (S.T split
    # across both HWDGE queues); chunk pairs interleave at step granularity;
    # pair 1's B tiles are woven into pair 0's final-step slots so the PE
    # never starves at the pair boundary.
    emit_th()
    emit_we()
    x_phase(0)
    x_phase(1)
    emit_thbc()
    emit_th_derive()
    b_phase(0, head_split=True)
    emit_st()
    b_phase(1)
    x_phase(2, dma_eng=nc.scalar)
    x_phase(3, dma_eng=nc.scalar)
    for _ in range(T_STEPS - 1):
        step(0)
        step(1)
    bts[2] = []
    zts[2] = []
    bts[3] = []
    zts[3] = []
    for idx in range(CHUNK // 128 * JHN):
        final_tile(0, idx)
        final_tile(1, idx)
        cb = 2 if idx < 4 else 3
        b_tile(cb, 2 * (idx % 4))
        b_tile(cb, 2 * (idx % 4) + 1)
    for _ in range(T_STEPS - 1):
        step(2)
        step(3)
    for idx in range(CHUNK // 128 * JHN):
        final_tile(2, idx)
        final_tile(3, idx)


def build_nc():
    nc = bacc.Bacc("TRN2", target_bir_lowering=False, debug=False)
    XT = nc.dram_tensor("XT", [N_IN, B_CORE], FP32R, kind="ExternalInput")
    WeT = nc.dram_tensor("WeT", [N_IN, M], FP32R, kind="ExternalInput")
    ST = nc.dram_tensor("ST", [M, M], FP32R, kind="ExternalInput")
    theta = nc.dram_tensor("theta", [M], FP32, kind="ExternalInput")
    Z = nc.dram_tensor("Z", [B_CORE, M], FP32, kind="ExternalOutput")
    with tile.TileContext(nc) as tc:
        with ExitStack() as ctx:
            _emit(ctx, tc, XT.ap(), WeT.ap(), ST.ap(), theta.ap(), Z.ap())
    nc.compile()
    return nc


_NC_CACHE = None


def _get_nc():
    global _NC_CACHE
    if _NC_CACHE is None:
        _NC_CACHE = build_nc()
    return _NC_CACHE


def run(X, We, S, theta, trace=False, **trace_kwargs):
    nc = _get_nc()
    X = np.asarray(X, dtype=np.float32)
    WeT = np.ascontiguousarray(np.asarray(We, dtype=np.float32).T)
    ST = np.ascontiguousarray(np.asarray(S, dtype=np.float32).T)
    theta = np.ascontiguousarray(np.asarray(theta, dtype=np.float32))
    in_maps = [
        {
            "XT": np.ascontiguousarray(X[i * B_CORE : (i + 1) * B_CORE].T),
            "WeT": WeT,
            "ST": ST,
            "theta": theta,
        }
        for i in range(N_CORES)
    ]
    # The PJRT compile callback can fail transiently ("CallFunctionObjArgs");
    # a retry in the same process succeeds.
    last_err = None
    for _attempt in range(3):
        try:
            res = bass_utils.run_bass_kernel_spmd(
                nc, in_maps, list(range(N_CORES)), trace=trace, **trace_kwargs
            )
            break
        except Exception as e:  # noqa: BLE001
            last_err = e
            time.sleep(2.0)
    else:
        raise last_err
    Z = np.concatenate([res.results[i]["Z"] for i in range(N_CORES)], axis=0)
    return Z.astype(np.float32, copy=False), res


def kernel(X, We, S, theta):
    Z, _ = run(X, We, S, theta, trace=False)
    return Z



# revision 3
# speedup vs baseline: 1.0514x; 1.0514x over previous
"""Trainium2 Bass kernel for LISTA (nn_LISTA_37976100831401).

Data-parallel sharding: batch 16384 -> 8 NeuronCores x 2048 rows.
We / S / theta are replicated on every core; no cross-device communication.

Per-core algorithm:
  B  = X @ We.T                 (2048, 1024)
  Z0 = soft(B);  Z_{t+1} = soft(B + Z_t @ S.T), t = 0..15
  soft(x) = relu(x - theta) - relu(-x - theta)

All matmul operands are bf16 (PSUM accumulation stays fp32).  On TRN2 the
measured back-to-back N=512 matmul period is 216 ns for bf16 vs 227 ns for
fp32r (the fp32r weight path costs ~13 ns/matmul extra), so bf16 runs the
2048x1024x1024 step matmuls at the PE streaming roofline.  Numerically the
bf16 pipeline lands at ~7e-3 relative error (gate is 2e-2): quantization
noise (~0.4%/step) does not compound destructively through the 16
soft-threshold steps.

Everything stays in the [feature, batch] orientation for all 16 steps:
  C.T = S @ Z.T + B.T  via  psum[j,b] += ST[k][:, j128].T @ ZT[k][:, b512]
so the matmul OUTPUT layout equals the INPUT layout of the next step.  The
device writes Z.T ([1024, 2048] per core) and the host transposes while
gathering -- this removes the baseline's flipped final step that re-derived
a batch-major layout by re-accumulating X@We.T on the PE.

The four 512-column batch chunks advance round-robin (c0..c3 per step), so
while chunk c's PSUM groups drain through DVE/ACT the PE streams the other
three chunks' matmuls (~41 us of cover for a ~3 us chain).

Opening: ~24 throwaway bf16 matmuls on memset tiles start as soon as the PE
engine boots, warming the HAM clock gate (1.2 -> 2.4 GHz takes ~3.4 us of
sustained PE activity) while the input DMAs run; S.T streams on the scalar
ring concurrently with We/X on the sync ring so the first step round never
waits on S.
"""

import time
from contextlib import ExitStack

import numpy as np
import ml_dtypes

import concourse.bacc as bacc
import concourse.mybir as mybir
import concourse.tile as tile
from concourse import bass_utils
from concourse.masks import make_identity

FP32 = mybir.dt.float32
BF16 = mybir.dt.bfloat16
AL = mybir.AluOpType
AF = mybir.ActivationFunctionType

N_CORES = 8
B_TOTAL, N_IN, M = 16384, 256, 1024
B_CORE = B_TOTAL // N_CORES  # 2048
T_STEPS = 16                 # scan length in the reference
CHUNK = 512                  # batch columns per PSUM group (= bank / max N)
N_CHUNKS = B_CORE // CHUNK   # 4
KT = M // 128                # 8 feature tiles of 128
NT = N_IN // 128             # 2 input-feature tiles
N_WARM = 24                  # HAM warmup matmuls


def _emit(ctx: ExitStack, tc: tile.TileContext, XT, WeT, ST, theta, ZT):
    nc = tc.nc

    const_pool = ctx.enter_context(tc.tile_pool(name="const", bufs=1))
    psum_pool = ctx.enter_context(tc.tile_pool(name="psum", bufs=1, space="PSUM"))
    xt_pool = ctx.enter_context(tc.tile_pool(name="xt", bufs=1))
    bt_pool = ctx.enter_context(tc.tile_pool(name="bt", bufs=1))
    zt_pool = ctx.enter_context(tc.tile_pool(name="zt", bufs=1))
    tmp_pool = ctx.enter_context(tc.tile_pool(name="tmp", bufs=1))
    out_pool = ctx.enter_context(tc.tile_pool(name="zout", bufs=1))

    # ---- constants -------------------------------------------------------
    ident = const_pool.tile([128, 128], FP32, name="ident")
    warm_a = const_pool.tile([128, 128], BF16, name="warm_a")
    warm_m = const_pool.tile([128, CHUNK], BF16, name="warm_m")
    th = const_pool.tile([128, KT], FP32, name="th")
    nth = const_pool.tile([128, KT], FP32, name="nth")
    wet = [const_pool.tile([128, M], BF16, name=f"wet{nt}") for nt in range(NT)]
    st = [const_pool.tile([128, M], BF16, name=f"st{kt}") for kt in range(KT)]
    th_row = const_pool.tile([1, M], FP32, name="th_row")
    thbc = const_pool.tile([128, M], FP32, name="thbc")
    ones_col = const_pool.tile([1, 128], FP32, name="ones_col")

    def emit_warmup():
        # PE work with zero DMA dependencies: warms the HAM clock gate while
        # inputs stream in.  Results are never read.
        make_identity(nc, ident[:])
        nc.gpsimd.memset(warm_a[:], 1.0)
        nc.gpsimd.memset(warm_m[:], 0.5)
        for i in range(N_WARM):
            pw = psum_pool.tile([128, CHUNK], FP32, name="pw", tag="tp", bufs=2)
            nc.tensor.matmul(pw[:], warm_a[:], warm_m[:], start=True, stop=True)

    def emit_dma_in():
        # scalar ring: theta row (4KB) then S.T (2MB); sync ring: We.T then
        # the four X.T chunk slabs in consumption order.
        nc.scalar.dma_start(th_row[:], theta.rearrange("(o m) -> o m", o=1))
        for kt in range(KT):
            nc.scalar.dma_start(st[kt][:], ST[kt * 128 : (kt + 1) * 128, :])
        for nt in range(NT):
            nc.sync.dma_start(wet[nt][:], WeT[nt * 128 : (nt + 1) * 128, :])

    xts = {}  # chunk -> [NT] tiles [128, CHUNK]

    def x_phase(c):
        xts[c] = [
            xt_pool.tile([128, CHUNK], BF16, name=f"xt{nt}", tag=f"xt{nt}", bufs=4)
            for nt in range(NT)
        ]
        for nt in range(NT):
            nc.sync.dma_start(
                xts[c][nt][:],
                XT[nt * 128 : (nt + 1) * 128, c * CHUNK : (c + 1) * CHUNK],
            )

    def emit_th():
        # thbc[p, j] = theta[j] via a K=1 ones-matmul, then th[p, jt] =
        # theta[jt*128 + p] by transposing each 128-wide slab (every column
        # of the transpose is the wanted theta block; copy out column 0).
        nc.gpsimd.memset(ones_col[:], 1.0)
        for jh in range(M // CHUNK):
            pbc = psum_pool.tile([128, CHUNK], FP32, name="pbc", tag="tp", bufs=2)
            nc.tensor.matmul(
                pbc[:], ones_col[:], th_row[:, jh * CHUNK : (jh + 1) * CHUNK],
                start=True, stop=True,
            )
            nc.vector.tensor_copy(thbc[:, jh * CHUNK : (jh + 1) * CHUNK], pbc[:])
        for jt in range(KT):
            pt = psum_pool.tile([128, 128], FP32, name="ptth", tag="tp", bufs=2)
            nc.tensor.transpose(
                pt[:], thbc[:, jt * 128 : (jt + 1) * 128], ident[:]
            )
            nc.vector.tensor_copy(th[:, jt : jt + 1], pt[:, 0:1])
        nc.vector.tensor_scalar_mul(nth[:], th[:], -1.0)

    # ---- per-chunk state -------------------------------------------------
    bts = {}  # chunk -> [KT] tiles [128, CHUNK] bf16  (B.T slab)
    zts = {}  # chunk -> [KT] tiles [128, CHUNK] bf16  (current Z.T)

    def b_phase(c):
        # B.T = We @ X.T ; Z0 = soft(B).  Two flavors balance the ACT/DVE
        # queues across chunks (each tile needs 3 post-ops: B copy + soft).
        bts[c] = []
        zts[c] = []
        for jt in range(KT):
            ps = psum_pool.tile([128, CHUNK], FP32, name="psb", tag="mm", bufs=6)
            for nt in range(NT):
                nc.tensor.matmul(
                    ps[:],
                    wet[nt][:, jt * 128 : (jt + 1) * 128],
                    xts[c][nt][:],
                    start=(nt == 0),
                    stop=(nt == NT - 1),
                )
            btile = bt_pool.tile(
                [128, CHUNK], BF16, name="btile", tag=f"bt{jt}", bufs=4
            )
            z0 = zt_pool.tile([128, CHUNK], BF16, name="z0", tag=f"zt{jt}", bufs=5)
            if c % 2 == 0:
                # ACT computes soft via two relus; DVE does the B copy.
                af = tmp_pool.tile([128, CHUNK], BF16, name="afb", tag="af", bufs=3)
                nc.scalar.activation(
                    af[:], ps[:], AF.Relu, bias=nth[:, jt : jt + 1], scale=1.0
                )
                df = tmp_pool.tile([128, CHUNK], BF16, name="dfb", tag="df", bufs=3)
                nc.scalar.activation(
                    df[:], ps[:], AF.Relu, bias=nth[:, jt : jt + 1], scale=-1.0
                )
                nc.vector.tensor_copy(btile[:], ps[:])
                nc.vector.tensor_sub(z0[:], af[:], df[:])
            else:
                # DVE computes soft via clamp; ACT does the B copy.
                t1 = tmp_pool.tile([128, CHUNK], FP32, name="t1b", tag="cf", bufs=3)
                nc.vector.tensor_scalar(
                    t1[:], ps[:], nth[:, jt : jt + 1], th[:, jt : jt + 1],
                    op0=AL.max, op1=AL.min,
                )
                nc.scalar.copy(btile[:], ps[:])
                nc.vector.tensor_sub(z0[:], ps[:], t1[:])
            bts[c].append(btile)
            zts[c].append(z0)

    def step(c):
        # Z <- soft(B + Z @ S.T), in the [j, b] orientation.
        zcur = zts[c]
        znew = []
        for jt in range(KT):
            ps = psum_pool.tile([128, CHUNK], FP32, name="pss", tag="mm", bufs=6)
            for kt in range(KT):
                nc.tensor.matmul(
                    ps[:],
                    st[kt][:, jt * 128 : (jt + 1) * 128],
                    zcur[kt][:],
                    start=(kt == 0),
                    stop=(kt == KT - 1),
                )
            ct = tmp_pool.tile([128, CHUNK], BF16, name="ct", tag="ct", bufs=3)
            nc.vector.tensor_add(ct[:], ps[:], bts[c][jt][:])
            af = tmp_pool.tile([128, CHUNK], BF16, name="afs", tag="af", bufs=3)
            nc.scalar.activation(
                af[:], ct[:], AF.Relu, bias=nth[:, jt : jt + 1], scale=1.0
            )
            df = tmp_pool.tile([128, CHUNK], BF16, name="dfs", tag="df", bufs=3)
            nc.scalar.activation(
                df[:], ct[:], AF.Relu, bias=nth[:, jt : jt + 1], scale=-1.0
            )
            zn = zt_pool.tile([128, CHUNK], BF16, name="zn", tag=f"zt{jt}", bufs=5)
            nc.vector.tensor_sub(zn[:], af[:], df[:])
            znew.append(zn)
        zts[c] = znew

    def final_step(c):
        # Last step keeps fp32 all the way to the output tile; Z.T DMAs out
        # row-contiguous (the host transposes while gathering).
        zcur = zts[c]
        for jt in range(KT):
            ps = psum_pool.tile([128, CHUNK], FP32, name="psf", tag="mm", bufs=6)
            for kt in range(KT):
                nc.tensor.matmul(
                    ps[:],
                    st[kt][:, jt * 128 : (jt + 1) * 128],
                    zcur[kt][:],
                    start=(kt == 0),
                    stop=(kt == KT - 1),
                )
            cf = tmp_pool.tile([128, CHUNK], FP32, name="cf", tag="cf", bufs=3)
            nc.vector.tensor_add(cf[:], ps[:], bts[c][jt][:])
            af = tmp_pool.tile([128, CHUNK], FP32, name="aff", tag="af", bufs=3)
            nc.scalar.activation(
                af[:], cf[:], AF.Relu, bias=nth[:, jt : jt + 1], scale=1.0
            )
            df = tmp_pool.tile([128, CHUNK], FP32, name="dff", tag="df", bufs=3)
            nc.scalar.activation(
                df[:], cf[:], AF.Relu, bias=nth[:, jt : jt + 1], scale=-1.0
            )
            zo = out_pool.tile([128, CHUNK], FP32, name="zo", tag="zo", bufs=4)
            nc.vector.tensor_sub(zo[:], af[:], df[:])
            dma_eng = nc.sync if jt % 2 == 0 else nc.scalar
            dma_eng.dma_start(
                ZT[jt * 128 : (jt + 1) * 128, c * CHUNK : (c + 1) * CHUNK], zo[:]
            )

    # ---- emission schedule ----------------------------------------------
    emit_warmup()
    emit_dma_in()
    for c in range(N_CHUNKS):
        x_phase(c)
    emit_th()
    for c in range(N_CHUNKS):
        b_phase(c)
    for _ in range(T_STEPS - 1):
        for c in range(N_CHUNKS):
            step(c)
    for c in range(N_CHUNKS):
        final_step(c)


def build_nc():
    nc = bacc.Bacc("TRN2", target_bir_lowering=False, debug=False)
    XT = nc.dram_tensor("XT", [N_IN, B_CORE], BF16, kind="ExternalInput")
    WeT = nc.dram_tensor("WeT", [N_IN, M], BF16, kind="ExternalInput")
    ST = nc.dram_tensor("ST", [M, M], BF16, kind="ExternalInput")
    theta = nc.dram_tensor("theta", [M], FP32, kind="ExternalInput")
    ZT = nc.dram_tensor("ZT", [M, B_CORE], FP32, kind="ExternalOutput")
    with tile.TileContext(nc) as tc:
        with ExitStack() as ctx:
            _emit(ctx, tc, XT.ap(), WeT.ap(), ST.ap(), theta.ap(), ZT.ap())
    nc.compile()
    return nc


_NC_CACHE = None


def _get_nc():
    global _NC_CACHE
    if _NC_CACHE is None:
        _NC_CACHE = build_nc()
    return _NC_CACHE


def make_in_maps(X, We, S, theta):
    X = np.asarray(X, dtype=np.float32)
    WeT = np.ascontiguousarray(np.asarray(We, dtype=np.float32).T).astype(
        ml_dtypes.bfloat16
    )
    ST = np.ascontiguousarray(np.asarray(S, dtype=np.float32).T).astype(
        ml_dtypes.bfloat16
    )
    theta = np.ascontiguousarray(np.asarray(theta, dtype=np.float32))
    return [
        {
            "XT": np.ascontiguousarray(X[i * B_CORE : (i + 1) * B_CORE].T).astype(
                ml_dtypes.bfloat16
            ),
            "WeT": WeT,
            "ST": ST,
            "theta": theta,
        }
        for i in range(N_CORES)
    ]


def gather_out(results):
    return np.concatenate(
        [
            np.asarray(results[i]["ZT"], dtype=np.float32).T
            for i in range(N_CORES)
        ],
        axis=0,
    )


def run(X, We, S, theta, trace=False, **trace_kwargs):
    nc = _get_nc()
    in_maps = make_in_maps(X, We, S, theta)
    # The PJRT compile callback can fail transiently ("CallFunctionObjArgs");
    # a retry in the same process succeeds.
    last_err = None
    for _attempt in range(3):
        try:
            res = bass_utils.run_bass_kernel_spmd(
                nc, in_maps, list(range(N_CORES)), trace=trace, **trace_kwargs
            )
            break
        except Exception as e:  # noqa: BLE001
            last_err = e
            time.sleep(2.0)
    else:
        raise last_err
    Z = gather_out(res.results)
    return Z.astype(np.float32, copy=False), res


def kernel(X, We, S, theta):
    Z, _ = run(X, We, S, theta, trace=False)
    return Z


# revision 7
# speedup vs baseline: 1.0636x; 1.0116x over previous
"""Trainium2 Bass kernel for LISTA (nn_LISTA_37976100831401).

Data-parallel sharding: batch 16384 -> 8 NeuronCores x 2048 rows.
We / S / theta are replicated on every core; no cross-device communication.

Per-core algorithm:
  B  = X @ We.T                 (2048, 1024)
  Z0 = soft(B);  Z_{t+1} = soft(B + Z_t @ S.T), t = 0..15
  soft(x) = relu(x - theta) - relu(-x - theta)

All matmul operands are bf16 (PSUM accumulation stays fp32).  On TRN2 the
measured back-to-back N=512 matmul period is 216 ns for bf16 vs 227 ns for
fp32r (the fp32r weight path costs ~13 ns/matmul extra), so bf16 runs the
2048x1024x1024 step matmuls at the PE streaming roofline.  Numerically the
bf16 pipeline lands at ~7e-3 relative error (gate is 2e-2): quantization
noise (~0.4%/step) does not compound destructively through the 16
soft-threshold steps.

Everything stays in the [feature, batch] orientation for all 16 steps:
  C.T = S @ Z.T + B.T  via  psum[j,b] += ST[k][:, j128].T @ ZT[k][:, b512]
so the matmul OUTPUT layout equals the INPUT layout of the next step.  The
device writes Z.T ([1024, 2048] per core) and the host transposes while
gathering -- this removes the baseline's flipped final step that re-derived
a batch-major layout by re-accumulating X@We.T on the PE.

The four 512-column batch chunks advance round-robin (c0..c3 per step), so
while chunk c's PSUM groups drain through DVE/ACT the PE streams the other
three chunks' matmuls (~41 us of cover for a ~3 us chain).

Opening: ~24 throwaway bf16 matmuls on memset tiles start as soon as the PE
engine boots, warming the HAM clock gate (1.2 -> 2.4 GHz takes ~3.4 us of
sustained PE activity) while the input DMAs run; S.T streams on the scalar
ring concurrently with We/X on the sync ring so the first step round never
waits on S.
"""

import time
from contextlib import ExitStack

import numpy as np
import ml_dtypes

import concourse.bacc as bacc
import concourse.mybir as mybir
import concourse.tile as tile
from concourse import bass_utils
from concourse.masks import make_identity

FP32 = mybir.dt.float32
BF16 = mybir.dt.bfloat16
AL = mybir.AluOpType
AF = mybir.ActivationFunctionType

N_CORES = 8
B_TOTAL, N_IN, M = 16384, 256, 1024
B_CORE = B_TOTAL // N_CORES  # 2048
T_STEPS = 16                 # scan length in the reference
CHUNK = 512                  # batch columns per PSUM group (= bank / max N)
N_CHUNKS = B_CORE // CHUNK   # 4
KT = M // 128                # 8 feature tiles of 128
NT = N_IN // 128             # 2 input-feature tiles
N_WARM = 16                  # HAM warmup matmuls
WARM_N = 256                 # moving width of warmup matmuls


def _emit(ctx: ExitStack, tc: tile.TileContext, XT, WeT, ST, theta, ZT):
    nc = tc.nc

    const_pool = ctx.enter_context(tc.tile_pool(name="const", bufs=1))
    psum_pool = ctx.enter_context(tc.tile_pool(name="psum", bufs=1, space="PSUM"))
    xt_pool = ctx.enter_context(tc.tile_pool(name="xt", bufs=1))
    bt_pool = ctx.enter_context(tc.tile_pool(name="bt", bufs=1))
    zt_pool = ctx.enter_context(tc.tile_pool(name="zt", bufs=1))
    tmp_pool = ctx.enter_context(tc.tile_pool(name="tmp", bufs=1))
    out_pool = ctx.enter_context(tc.tile_pool(name="zout", bufs=1))

    # ---- constants -------------------------------------------------------
    ident = const_pool.tile([128, 128], FP32, name="ident")
    warm_a = const_pool.tile([128, 128], BF16, name="warm_a")
    warm_m = const_pool.tile([128, WARM_N], BF16, name="warm_m")
    th = const_pool.tile([128, KT], FP32, name="th")
    nth = const_pool.tile([128, KT], FP32, name="nth")
    wet = [const_pool.tile([128, M], BF16, name=f"wet{nt}") for nt in range(NT)]
    st = [const_pool.tile([128, M], BF16, name=f"st{kt}") for kt in range(KT)]
    th_row = const_pool.tile([1, M], FP32, name="th_row")
    thbc = const_pool.tile([128, M], FP32, name="thbc")
    ones_col = const_pool.tile([1, 128], FP32, name="ones_col")

    def emit_warmup():
        # PE work with zero DMA dependencies: warms the HAM clock gate while
        # inputs stream in.  Results are never read.
        make_identity(nc, ident[:])
        nc.gpsimd.memset(warm_a[:], 1.0)
        nc.gpsimd.memset(warm_m[:], 0.5)
        for i in range(N_WARM):
            pw = psum_pool.tile([128, WARM_N], FP32, name="pw", tag="tp", bufs=2)
            nc.tensor.matmul(pw[:], warm_a[:], warm_m[:], start=True, stop=True)

    def emit_dma_in():
        # scalar ring: theta row (4KB) then S.T (2MB); sync ring: We.T then
        # the four X.T chunk slabs in consumption order.
        nc.scalar.dma_start(th_row[:], theta.rearrange("(o m) -> o m", o=1))
        for kt in range(KT):
            nc.scalar.dma_start(st[kt][:], ST[kt * 128 : (kt + 1) * 128, :])
        for nt in range(NT):
            nc.sync.dma_start(wet[nt][:], WeT[nt * 128 : (nt + 1) * 128, :])

    xts = {}  # chunk -> [NT] tiles [128, CHUNK]

    def x_phase(c):
        xts[c] = [
            xt_pool.tile([128, CHUNK], BF16, name=f"xt{nt}", tag=f"xt{nt}", bufs=4)
            for nt in range(NT)
        ]
        for nt in range(NT):
            nc.sync.dma_start(
                xts[c][nt][:],
                XT[nt * 128 : (nt + 1) * 128, c * CHUNK : (c + 1) * CHUNK],
            )

    def emit_th():
        # thbc[p, j] = theta[j] via a K=1 ones-matmul, then th[p, jt] =
        # theta[jt*128 + p] by transposing each 128-wide slab (every column
        # of the transpose is the wanted theta block; copy out column 0).
        nc.gpsimd.memset(ones_col[:], 1.0)
        for jh in range(M // CHUNK):
            pbc = psum_pool.tile([128, CHUNK], FP32, name="pbc", tag="tp", bufs=2)
            nc.tensor.matmul(
                pbc[:], ones_col[:], th_row[:, jh * CHUNK : (jh + 1) * CHUNK],
                start=True, stop=True,
            )
            nc.vector.tensor_copy(thbc[:, jh * CHUNK : (jh + 1) * CHUNK], pbc[:])
        for jt in range(KT):
            pt = psum_pool.tile([128, 128], FP32, name="ptth", tag="tp", bufs=2)
            nc.tensor.transpose(
                pt[:], thbc[:, jt * 128 : (jt + 1) * 128], ident[:]
            )
            nc.vector.tensor_copy(th[:, jt : jt + 1], pt[:, 0:1])
        nc.vector.tensor_scalar_mul(nth[:], th[:], -1.0)

    # ---- per-chunk state -------------------------------------------------
    bts = {}  # chunk -> [KT] tiles [128, CHUNK] bf16  (B.T slab)
    zts = {}  # chunk -> [KT] tiles [128, CHUNK] bf16  (current Z.T)

    def b_phase(c):
        # B.T = We @ X.T ; Z0 = soft(B).  Each b-group is only ~432 ns of PE
        # work (K=256), so PSUM bank turnaround gates the PE here: the bank
        # must be read exactly once and released fast.  Two parallel
        # half-copies (ACT + DVE) move B to SBUF in ~400 ns; Z0 is then
        # computed from the bf16 B tile:
        #   soft(B) = relu(B - th) + min(B + th, 0)
        bts[c] = []
        zts[c] = []
        for jt in range(KT):
            ps = psum_pool.tile([128, CHUNK], FP32, name="psb", tag="mm", bufs=6)
            for nt in range(NT):
                nc.tensor.matmul(
                    ps[:],
                    wet[nt][:, jt * 128 : (jt + 1) * 128],
                    xts[c][nt][:],
                    start=(nt == 0),
                    stop=(nt == NT - 1),
                )
            btile = bt_pool.tile(
                [128, CHUNK], BF16, name="btile", tag=f"bt{jt}", bufs=4
            )
            half = CHUNK // 2
            nc.scalar.copy(btile[:, :half], ps[:, :half])
            nc.vector.tensor_copy(btile[:, half:], ps[:, half:])
            z0 = zt_pool.tile([128, CHUNK], BF16, name="z0", tag=f"zt{jt}", bufs=5)
            af = tmp_pool.tile([128, CHUNK], BF16, name="afb", tag="af", bufs=3)
            nc.scalar.activation(
                af[:], btile[:], AF.Relu, bias=nth[:, jt : jt + 1], scale=1.0
            )
            df = tmp_pool.tile([128, CHUNK], BF16, name="dfb", tag="df", bufs=3)
            nc.vector.tensor_scalar(
                df[:], btile[:], th[:, jt : jt + 1], 0.0, op0=AL.add, op1=AL.min
            )
            nc.vector.tensor_add(z0[:], af[:], df[:])
            bts[c].append(btile)
            zts[c].append(z0)

    def step(c):
        # Z <- soft(B + Z @ S.T), in the [j, b] orientation.
        zcur = zts[c]
        znew = []
        for jt in range(KT):
            ps = psum_pool.tile([128, CHUNK], FP32, name="pss", tag="mm", bufs=6)
            for kt in range(KT):
                nc.tensor.matmul(
                    ps[:],
                    st[kt][:, jt * 128 : (jt + 1) * 128],
                    zcur[kt][:],
                    start=(kt == 0),
                    stop=(kt == KT - 1),
                )
            ct = tmp_pool.tile([128, CHUNK], BF16, name="ct", tag="ct", bufs=3)
            nc.vector.tensor_add(ct[:], ps[:], bts[c][jt][:])
            af = tmp_pool.tile([128, CHUNK], BF16, name="afs", tag="af", bufs=3)
            nc.scalar.activation(
                af[:], ct[:], AF.Relu, bias=nth[:, jt : jt + 1], scale=1.0
            )
            df = tmp_pool.tile([128, CHUNK], BF16, name="dfs", tag="df", bufs=3)
            nc.scalar.activation(
                df[:], ct[:], AF.Relu, bias=nth[:, jt : jt + 1], scale=-1.0
            )
            zn = zt_pool.tile([128, CHUNK], BF16, name="zn", tag=f"zt{jt}", bufs=5)
            nc.vector.tensor_sub(zn[:], af[:], df[:])
            znew.append(zn)
        zts[c] = znew

    def final_step(c):
        # Last step keeps fp32 all the way to the output tile; Z.T DMAs out
        # row-contiguous (the host transposes while gathering).
        zcur = zts[c]
        for jt in range(KT):
            ps = psum_pool.tile([128, CHUNK], FP32, name="psf", tag="mm", bufs=6)
            for kt in range(KT):
                nc.tensor.matmul(
                    ps[:],
                    st[kt][:, jt * 128 : (jt + 1) * 128],
                    zcur[kt][:],
                    start=(kt == 0),
                    stop=(kt == KT - 1),
                )
            cf = tmp_pool.tile([128, CHUNK], FP32, name="cf", tag="cf", bufs=3)
            nc.vector.tensor_add(cf[:], ps[:], bts[c][jt][:])
            af = tmp_pool.tile([128, CHUNK], FP32, name="aff", tag="af", bufs=3)
            nc.scalar.activation(
                af[:], cf[:], AF.Relu, bias=nth[:, jt : jt + 1], scale=1.0
            )
            df = tmp_pool.tile([128, CHUNK], FP32, name="dff", tag="df", bufs=3)
            nc.scalar.activation(
                df[:], cf[:], AF.Relu, bias=nth[:, jt : jt + 1], scale=-1.0
            )
            zo = out_pool.tile([128, CHUNK], FP32, name="zo", tag="zo", bufs=4)
            nc.vector.tensor_sub(zo[:], af[:], df[:])
            dma_eng = nc.sync if jt % 2 == 0 else nc.scalar
            dma_eng.dma_start(
                ZT[jt * 128 : (jt + 1) * 128, c * CHUNK : (c + 1) * CHUNK], zo[:]
            )

    # ---- emission schedule ----------------------------------------------
    emit_warmup()
    emit_dma_in()
    for c in range(N_CHUNKS):
        x_phase(c)
    emit_th()
    for c in range(N_CHUNKS):
        b_phase(c)
    for _ in range(T_STEPS - 1):
        for c in range(N_CHUNKS):
            step(c)
    for c in range(N_CHUNKS):
        final_step(c)


def build_nc():
    nc = bacc.Bacc("TRN2", target_bir_lowering=False, debug=False)
    XT = nc.dram_tensor("XT", [N_IN, B_CORE], BF16, kind="ExternalInput")
    WeT = nc.dram_tensor("WeT", [N_IN, M], BF16, kind="ExternalInput")
    ST = nc.dram_tensor("ST", [M, M], BF16, kind="ExternalInput")
    theta = nc.dram_tensor("theta", [M], FP32, kind="ExternalInput")
    ZT = nc.dram_tensor("ZT", [M, B_CORE], FP32, kind="ExternalOutput")
    with tile.TileContext(nc) as tc:
        with ExitStack() as ctx:
            _emit(ctx, tc, XT.ap(), WeT.ap(), ST.ap(), theta.ap(), ZT.ap())
    nc.compile()
    return nc


_NC_CACHE = None


def _get_nc():
    global _NC_CACHE
    if _NC_CACHE is None:
        _NC_CACHE = build_nc()
    return _NC_CACHE


def make_in_maps(X, We, S, theta):
    X = np.asarray(X, dtype=np.float32)
    WeT = np.ascontiguousarray(np.asarray(We, dtype=np.float32).T).astype(
        ml_dtypes.bfloat16
    )
    ST = np.ascontiguousarray(np.asarray(S, dtype=np.float32).T).astype(
        ml_dtypes.bfloat16
    )
    theta = np.ascontiguousarray(np.asarray(theta, dtype=np.float32))
    return [
        {
            "XT": np.ascontiguousarray(X[i * B_CORE : (i + 1) * B_CORE].T).astype(
                ml_dtypes.bfloat16
            ),
            "WeT": WeT,
            "ST": ST,
            "theta": theta,
        }
        for i in range(N_CORES)
    ]


def gather_out(results):
    return np.concatenate(
        [
            np.asarray(results[i]["ZT"], dtype=np.float32).T
            for i in range(N_CORES)
        ],
        axis=0,
    )


def run(X, We, S, theta, trace=False, **trace_kwargs):
    nc = _get_nc()
    in_maps = make_in_maps(X, We, S, theta)
    # The PJRT compile callback can fail transiently ("CallFunctionObjArgs");
    # a retry in the same process succeeds.
    last_err = None
    for _attempt in range(3):
        try:
            res = bass_utils.run_bass_kernel_spmd(
                nc, in_maps, list(range(N_CORES)), trace=trace, **trace_kwargs
            )
            break
        except Exception as e:  # noqa: BLE001
            last_err = e
            time.sleep(2.0)
    else:
        raise last_err
    Z = gather_out(res.results)
    return Z.astype(np.float32, copy=False), res


def kernel(X, We, S, theta):
    Z, _ = run(X, We, S, theta, trace=False)
    return Z
